# revision 1
# baseline (speedup 1.0000x reference)
"""ConvSNN forward on 8 Trainium2 NeuronCores.

Math (matches the reference nn.Module):
    post_current = conv2d(x, w, 3x3, stride 1, pad 1)   # [B, 256, 56, 56]
    spikes       = (post_current >= 1.0) ? 1.0 : 0.0

Strategy:
  - Data parallel: 32 images -> 8 cores x 4 images. Weight replicated.
  - Per image: x [128, 56, 56] lives in SBUF as a zero-padded [128, 58, 58]
    tile (partition dim = C_in = 128). The 3x3 conv is 9 accumulating
    matmuls per output tile, one per kernel tap, each reading a shifted
    window of the padded image (free dims stride over the 58x58 layout).
  - C_out = 256 -> two 128-row halves (PSUM partition limit).
  - Output pixels tiled 8 rows (448 px) per PSUM bank (<= 512 fp32).
  - Spike threshold: tensor_scalar is_ge 1.0 (PSUM -> SBUF), DMA out.
"""

import numpy as np

B_FULL = 32
N_CORES = 8
B_LOCAL = B_FULL // N_CORES  # 4
C_IN = 128
C_OUT = 256
H = W = 56
KS = 3
HP, WP = H + 2, W + 2  # zero-padded image in SBUF
ROWS_PER_CHUNK = 8
N_CHUNKS = H // ROWS_PER_CHUNK  # 7

# matmul operand dtype: "fp32" (exact, 4 cyc/row) or "fp32r" (1 cyc/row at
# free-dim >= 256, reduced-precision PE path)
MODE = "fp32"
# set by test.py to get an NTFF profile (exec_time_ns) out of the run
PROFILE = False
LAST_RESULT = None

_PROG_CACHE = {}


def _build_program(mode):
    import concourse.bacc as bacc
    import concourse.mybir as mybir
    import concourse.tile as tile

    f32 = mybir.dt.float32
    mm_dt = {"fp32": f32, "fp32r": mybir.dt.float32r}[mode]

    nc = bacc.Bacc("TRN2", target_bir_lowering=False, debug=False,
                   num_devices=N_CORES)
    x_d = nc.dram_tensor("x", [B_LOCAL, C_IN, H, W], f32,
                         kind="ExternalInput").ap()
    w_d = nc.dram_tensor("w", [C_IN, KS * KS, C_OUT], f32,
                         kind="ExternalInput").ap()
    y_d = nc.dram_tensor("y", [B_LOCAL, C_OUT, H, W], f32,
                         kind="ExternalOutput").ap()

    with tile.TileContext(nc) as tc:
        with (
            tc.tile_pool(name="wpool", bufs=1) as wpool,
            tc.tile_pool(name="xpool", bufs=2) as xpool,
            tc.tile_pool(name="opool", bufs=4) as opool,
            tc.tile_pool(name="psum", bufs=8, space="PSUM") as pspool,
        ):
            w_sb = wpool.tile([C_IN, KS * KS, C_OUT], f32)
            nc.sync.dma_start(w_sb[:], w_d[:])

            for img in range(B_LOCAL):
                xt = xpool.tile([C_IN, HP, WP], f32, tag="x")
                # interior is overwritten by the DMA; only the 1-px border
                # needs zeroing
                nc.vector.memset(xt[:, 0, :], 0.0)
                nc.vector.memset(xt[:, H + 1, :], 0.0)
                nc.vector.memset(xt[:, 1:H + 1, 0], 0.0)
                nc.vector.memset(xt[:, 1:H + 1, W + 1], 0.0)
                nc.sync.dma_start(xt[:, 1:H + 1, 1:W + 1], x_d[img])

                for half in range(2):
                    for c in range(N_CHUNKS):
                        r0 = c * ROWS_PER_CHUNK
                        ps = pspool.tile([128, ROWS_PER_CHUNK, W], f32,
                                         tag="ps")
                        for k in range(KS * KS):
                            kh, kw = divmod(k, KS)
                            lhsT = w_sb[:, k, half * 128:(half + 1) * 128]
                            rhs = xt[:, r0 + kh:r0 + kh + ROWS_PER_CHUNK,
                                     kw:kw + W]
                            if mm_dt is not f32:
                                lhsT = lhsT.bitcast(mm_dt)
                                rhs = rhs.bitcast(mm_dt)
                            nc.tensor.matmul(ps[:], lhsT, rhs,
                                             start=(k == 0),
                                             stop=(k == KS * KS - 1))
                        ot = opool.tile([128, ROWS_PER_CHUNK, W], f32,
                                        tag="o")
                        nc.any.tensor_scalar(ot[:], ps[:], 1.0, None,
                                             mybir.AluOpType.is_ge)
                        nc.sync.dma_start(
                            y_d[img, half * 128:(half + 1) * 128,
                                r0:r0 + ROWS_PER_CHUNK, :],
                            ot[:])
    nc.compile()
    return nc


def _get_program(mode):
    if mode not in _PROG_CACHE:
        _PROG_CACHE[mode] = _build_program(mode)
    return _PROG_CACHE[mode]


def kernel(x, weight):
    global LAST_RESULT
    from concourse.bass_utils import run_bass_kernel_spmd

    x = np.ascontiguousarray(np.asarray(x), dtype=np.float32)
    w = np.asarray(weight, dtype=np.float32)
    # weight[o, c*9 + kh*3 + kw] -> w_sb[c, kh*3+kw, o] (lhsT layout:
    # partition dim = contraction C_in, free dim = C_out)
    w_sb = np.ascontiguousarray(
        w.reshape(C_OUT, C_IN, KS, KS).transpose(1, 2, 3, 0)
        .reshape(C_IN, KS * KS, C_OUT))

    nc = _get_program(MODE)
    in_maps = [
        {"x": x[i * B_LOCAL:(i + 1) * B_LOCAL], "w": w_sb}
        for i in range(N_CORES)
    ]
    res = run_bass_kernel_spmd(nc, in_maps, list(range(N_CORES)),
                               trace=PROFILE)
    LAST_RESULT = res
    return np.concatenate([res.results[i]["y"] for i in range(N_CORES)],
                          axis=0)


# revision 7
# speedup vs baseline: 1.0225x; 1.0225x over previous
"""ConvSNN forward on 8 Trainium2 NeuronCores.

Math (matches the reference nn.Module):
    post_current = conv2d(x, w, 3x3, stride 1, pad 1)   # [B, 256, 56, 56]
    spikes       = (post_current >= 1.0) ? 1.0 : 0.0

Strategy:
  - Data parallel: 32 images -> 8 cores x 4 images. Weight replicated.
  - Per image: x [128, 56, 56] lives in SBUF as a row-padded [128, 58, 56]
    tile (partition dim = C_in = 128; rows 0 and 57 are zeros, interior is
    one fully-contiguous DMA). The 3x3 conv is 9 accumulating matmuls per
    output tile, one per kernel tap, reading a shifted window.
  - Column taps (kw = +/-1) use width-55 matmuls that skip the border
    column instead of column padding: PSUM accumulation is per-element
    (has_written bits), so narrower taps accumulate into the interior only
    -- exact, as long as the first matmul of each group covers the full
    tile (the kw=0 taps do).
  - C_out = 256 -> two 128-row halves (PSUM partition limit).
  - Output pixels tiled 8 rows (448 px) per PSUM bank (<= 512 fp32).
  - Spike threshold: tensor_scalar is_ge 1.0 (PSUM -> SBUF), DMA out.
"""

import numpy as np

B_FULL = 32
N_CORES = 8
B_LOCAL = B_FULL // N_CORES  # 4
C_IN = 128
C_OUT = 256
H = W = 56
KS = 3
HP = H + 2  # row-padded image in SBUF
ROWS_PER_CHUNK = 8
N_CHUNKS = H // ROWS_PER_CHUNK  # 7

# matmul operand dtype: "fp32" (exact, 4 cyc/row) or "fp32r" (1 cyc/row at
# free-dim >= 256, reduced-precision PE path)
MODE = "fp32"
# set by test.py to get an NTFF profile (exec_time_ns) out of the run
PROFILE = False
LAST_RESULT = None

_PROG_CACHE = {}

# kernel taps ordered so the first matmul of every accumulation group is
# full-width (kw=1 = no column shift); narrower +/-1 column taps follow.
TAPS = [(0, 1), (1, 1), (2, 1), (0, 0), (1, 0), (2, 0), (0, 2), (1, 2),
        (2, 2)]


def _build_program(mode):
    import concourse.bacc as bacc
    import concourse.mybir as mybir
    import concourse.tile as tile

    f32 = mybir.dt.float32
    mm_dt = {"fp32": f32, "fp32r": mybir.dt.float32r}[mode]

    nc = bacc.Bacc("TRN2", target_bir_lowering=False, debug=False,
                   num_devices=N_CORES)
    x_d = nc.dram_tensor("x", [B_LOCAL, C_IN, H, W], f32,
                         kind="ExternalInput").ap()
    w_d = nc.dram_tensor("w", [C_IN, KS * KS, C_OUT], f32,
                         kind="ExternalInput").ap()
    z_d = nc.dram_tensor("z", [C_IN, W], f32, kind="ExternalInput").ap()
    y_d = nc.dram_tensor("y", [B_LOCAL, C_OUT, H, W], f32,
                         kind="ExternalOutput").ap()

    with tile.TileContext(nc) as tc:
        with (
            tc.tile_pool(name="wpool", bufs=1) as wpool,
            tc.tile_pool(name="xpool", bufs=2) as xpool,
            tc.tile_pool(name="opool", bufs=4) as opool,
            tc.tile_pool(name="psum", bufs=8, space="PSUM") as pspool,
        ):
            # fp32 -> fp32r casting DMAs must go through gpsimd (SWDGE)
            dma = nc.sync if mm_dt is f32 else nc.gpsimd
            w_sb = wpool.tile([C_IN, KS * KS, C_OUT], mm_dt)
            dma.dma_start(w_sb[:], w_d[:])

            for img in range(B_LOCAL):
                xt = xpool.tile([C_IN, HP, W], mm_dt, tag="x")
                dma.dma_start(xt[:, 0, :], z_d[:])
                dma.dma_start(xt[:, H + 1, :], z_d[:])
                dma.dma_start(xt[:, 1:H + 1, :], x_d[img])

                for half in range(2):
                    for c in range(N_CHUNKS):
                        r0 = c * ROWS_PER_CHUNK
                        ps = pspool.tile([128, ROWS_PER_CHUNK, W], f32,
                                         tag="ps")
                        for i, (kh, kw) in enumerate(TAPS):
                            k = kh * KS + kw
                            lhsT = w_sb[:, k, half * 128:(half + 1) * 128]
                            rows = xt[:, r0 + kh:r0 + kh + ROWS_PER_CHUNK, :]
                            if kw == 1:      # no column shift, full width
                                out_ap, rhs = ps[:], rows
                            elif kw == 0:    # reads col-1: skip out col 0
                                out_ap, rhs = ps[:, :, 1:], rows[:, :, :W - 1]
                            else:            # reads col+1: skip out col W-1
                                out_ap, rhs = ps[:, :, :W - 1], rows[:, :, 1:]
                            nc.tensor.matmul(out_ap, lhsT, rhs,
                                             start=(i == 0),
                                             stop=(i == len(TAPS) - 1))
                        ot = opool.tile([128, ROWS_PER_CHUNK, W], f32,
                                        tag="o")
                        nc.any.tensor_scalar(ot[:], ps[:], 1.0, None,
                                             mybir.AluOpType.is_ge)
                        nc.sync.dma_start(
                            y_d[img, half * 128:(half + 1) * 128,
                                r0:r0 + ROWS_PER_CHUNK, :],
                            ot[:])
    nc.compile()
    return nc


def _get_program(mode):
    if mode not in _PROG_CACHE:
        _PROG_CACHE[mode] = _build_program(mode)
    return _PROG_CACHE[mode]


def kernel(x, weight):
    global LAST_RESULT
    from concourse.bass_utils import run_bass_kernel_spmd

    x = np.ascontiguousarray(np.asarray(x), dtype=np.float32)
    w = np.asarray(weight, dtype=np.float32)
    # weight[o, c*9 + kh*3 + kw] -> w_sb[c, kh*3+kw, o] (lhsT layout:
    # partition dim = contraction C_in, free dim = C_out)
    w_sb = np.ascontiguousarray(
        w.reshape(C_OUT, C_IN, KS, KS).transpose(1, 2, 3, 0)
        .reshape(C_IN, KS * KS, C_OUT))
    zeros = np.zeros((C_IN, W), dtype=np.float32)

    nc = _get_program(MODE)
    in_maps = [
        {"x": x[i * B_LOCAL:(i + 1) * B_LOCAL], "w": w_sb, "z": zeros}
        for i in range(N_CORES)
    ]
    res = run_bass_kernel_spmd(nc, in_maps, list(range(N_CORES)),
                               trace=PROFILE)
    LAST_RESULT = res
    return np.concatenate([res.results[i]["y"] for i in range(N_CORES)],
                          axis=0)


# revision 10
# speedup vs baseline: 3.1336x; 3.0646x over previous
"""ConvSNN forward on 8 Trainium2 NeuronCores.

Math (matches the reference nn.Module):
    post_current = conv2d(x, w, 3x3, stride 1, pad 1)   # [B, 256, 56, 56]
    spikes       = (post_current >= 1.0) ? 1.0 : 0.0

Strategy:
  - Data parallel: 32 images -> 8 cores x 4 images. Weight replicated.
  - The host pre-pads each image into a flat [128, 1 + 58*57] layout:
    one leading zero guard, then 58 rows (zero row, 56 image rows, zero
    row) of 57 elements (56 cols + 1 zero separator col). A 3x3 tap
    (kh, kw) is then the [8, 56]-window (row stride 57) at flat offset
    kw + 57*(r0+kh): the separator/guard zeros supply the left/right
    conv padding. One fully contiguous DMA per image, no memsets, and
    every matmul is full width (innermost free count 56 is even, which
    the fp32r PE path requires).
  - Conv = 9 accumulating matmuls per PSUM tile (one per tap), weights
    stationary [C_in=128 x 128], moving operand = shifted image window.
  - C_out = 256 -> two 128-row halves (PSUM partition limit).
  - Output pixels tiled 8 rows (448 px) per PSUM bank (<= 512 fp32).
  - Spike threshold: tensor_scalar is_ge 1.0 (PSUM -> SBUF), DMA out.
"""

import numpy as np

B_FULL = 32
N_CORES = 8
B_LOCAL = B_FULL // N_CORES  # 4
C_IN = 128
C_OUT = 256
H = W = 56
KS = 3
WPAD = W + 1        # 56 cols + zero separator
HPAD = H + 2        # zero row, image, zero row
FLAT = 1 + HPAD * WPAD + 1  # leading/trailing zero guards for tap shifts
ROWS_PER_CHUNK = 8
N_CHUNKS = H // ROWS_PER_CHUNK  # 7

# matmul operand dtype: "fp32" (exact, 4 cyc/row) or "fp32r" (1 cyc/row at
# free-dim >= 256, reduced-precision PE path)
MODE = "fp32r"
# set by test.py to get an NTFF profile (exec_time_ns) out of the run
PROFILE = False
LAST_RESULT = None

_PROG_CACHE = {}


def _build_program(mode):
    import concourse.bacc as bacc
    import concourse.mybir as mybir
    import concourse.tile as tile

    f32 = mybir.dt.float32
    mm_dt = {"fp32": f32, "fp32r": mybir.dt.float32r}[mode]

    nc = bacc.Bacc("TRN2", target_bir_lowering=False, debug=False,
                   num_devices=N_CORES)
    x_d = nc.dram_tensor("x", [B_LOCAL, C_IN, FLAT], f32,
                         kind="ExternalInput").ap()
    w_d = nc.dram_tensor("w", [C_IN, KS * KS, C_OUT], f32,
                         kind="ExternalInput").ap()
    y_d = nc.dram_tensor("y", [B_LOCAL, C_OUT, H, W], f32,
                         kind="ExternalOutput").ap()

    with tile.TileContext(nc) as tc:
        with (
            tc.tile_pool(name="wpool", bufs=1) as wpool,
            tc.tile_pool(name="xpool", bufs=2) as xpool,
            tc.tile_pool(name="opool", bufs=4) as opool,
            tc.tile_pool(name="psum", bufs=8, space="PSUM") as pspool,
        ):
            # fp32 -> fp32r casting DMAs must go through gpsimd (SWDGE)
            dma = nc.sync if mm_dt is f32 else nc.gpsimd
            w_sb = wpool.tile([C_IN, KS * KS, C_OUT], mm_dt)
            dma.dma_start(w_sb[:], w_d[:])

            for img in range(B_LOCAL):
                xt = xpool.tile([C_IN, FLAT], mm_dt, tag="x")
                dma.dma_start(xt[:], x_d[img])
                # three column-shifted [HPAD, WPAD] views of the flat image
                views = [
                    xt[:, kw:kw + HPAD * WPAD]
                    .rearrange("p (r c) -> p r c", c=WPAD)
                    for kw in range(KS)
                ]

                for half in range(2):
                    for c in range(N_CHUNKS):
                        r0 = c * ROWS_PER_CHUNK
                        ps = pspool.tile([128, ROWS_PER_CHUNK, W], f32,
                                         tag="ps")
                        for k in range(KS * KS):
                            kh, kw = divmod(k, KS)
                            lhsT = w_sb[:, k, half * 128:(half + 1) * 128]
                            rhs = views[kw][:, r0 + kh:r0 + kh
                                            + ROWS_PER_CHUNK, :W]
                            nc.tensor.matmul(ps[:], lhsT, rhs,
                                             start=(k == 0),
                                             stop=(k == KS * KS - 1))
                        ot = opool.tile([128, ROWS_PER_CHUNK, W], f32,
                                        tag="o")
                        nc.any.tensor_scalar(ot[:], ps[:], 1.0, None,
                                             mybir.AluOpType.is_ge)
                        nc.sync.dma_start(
                            y_d[img, half * 128:(half + 1) * 128,
                                r0:r0 + ROWS_PER_CHUNK, :],
                            ot[:])
    nc.compile()
    return nc


def _get_program(mode):
    if mode not in _PROG_CACHE:
        _PROG_CACHE[mode] = _build_program(mode)
    return _PROG_CACHE[mode]


def _pad_images(x):
    """[B, C, 56, 56] -> flat guarded layout [B, C, FLAT] (see module doc)."""
    b = x.shape[0]
    xp = np.zeros((b, C_IN, FLAT), dtype=np.float32)
    view = xp[:, :, 1:-1].reshape(b, C_IN, HPAD, WPAD)
    view[:, :, 1:H + 1, :W] = x
    return xp


def kernel(x, weight):
    global LAST_RESULT
    from concourse.bass_utils import run_bass_kernel_spmd

    x = np.asarray(x, dtype=np.float32)
    w = np.asarray(weight, dtype=np.float32)
    # weight[o, c*9 + kh*3 + kw] -> w_sb[c, kh*3+kw, o] (lhsT layout:
    # partition dim = contraction C_in, free dim = C_out)
    w_sb = np.ascontiguousarray(
        w.reshape(C_OUT, C_IN, KS, KS).transpose(1, 2, 3, 0)
        .reshape(C_IN, KS * KS, C_OUT))
    xp = _pad_images(x)

    nc = _get_program(MODE)
    in_maps = [
        {"x": xp[i * B_LOCAL:(i + 1) * B_LOCAL], "w": w_sb}
        for i in range(N_CORES)
    ]
    res = run_bass_kernel_spmd(nc, in_maps, list(range(N_CORES)),
                               trace=PROFILE)
    LAST_RESULT = res
    return np.concatenate([res.results[i]["y"] for i in range(N_CORES)],
                          axis=0)
